# revision 1
# baseline (speedup 1.0000x reference)
"""Graph-Transformer message-passing kernel for 8 Trainium2 NeuronCores.

Strategy (1D dst-shard edge parallelism):
  - Nodes are split into 8 contiguous ranges; core c owns all edges whose dst
    falls in its range, so segment max/sum/aggregation are fully local.
  - Host groups each core's dst nodes into degree classes (W in {4,8,16,32,64}),
    pads each node's in-edge list to W slots (pad slots get bias -1e30 so they
    vanish in softmax), and lays nodes out in a per-core "perm" order so the
    on-device pipeline is fully regular.
  - Per layer, each core projects Q|K|V for its own nodes (PE), the q|v halves
    are AllGathered into a global qv table in DRAM, and each node-tile then
    indirect-DMA-gathers its W source rows, runs the masked edge softmax and
    weighted aggregation on DVE/ACT, and applies the output projection (PE).
  - 3 layers are fused in one NEFF; output rows return in perm order and the
    host inverts the permutation.
"""

import numpy as np

import concourse.bass as bass
import concourse.bacc as bacc
import concourse.mybir as mybir
import concourse.tile as tile
from concourse.bass import IndirectOffsetOnAxis
from concourse.masks import make_identity
from concourse.bass_utils import run_bass_kernel_spmd

NCORES = 8
L = 3
H = 8
D = 128
HD = D // H
SCALE = 1.0 / float(np.sqrt(HD))
NEG = -1.0e30
P = 128

FP = mybir.dt.float32
I32 = mybir.dt.int32
AX = mybir.AxisListType
OP = mybir.AluOpType


# ----------------------------------------------------------------------------
# Host-side layout
# ----------------------------------------------------------------------------

class Layout:
    pass


def build_layout(src, dst, n_nodes):
    """Group each core's dst nodes by degree class and build gather tables."""
    src = np.asarray(src).astype(np.int64)
    dst = np.asarray(dst).astype(np.int64)
    N = n_nodes
    chunk = (N + NCORES - 1) // NCORES

    deg = np.bincount(dst, minlength=N)
    order = np.argsort(dst, kind="stable")
    src_sorted = src[order]
    starts = np.zeros(N + 1, dtype=np.int64)
    np.cumsum(deg, out=starts[1:])

    max_deg = int(deg.max())
    w_all = [4, 8, 16, 32, 64]
    assert max_deg <= w_all[-1], f"max degree {max_deg} exceeds supported 64"
    classes = []
    lo = 0
    for w in w_all:
        sel = (deg > lo) & (deg <= w)
        if sel.any():
            classes.append(w)
        lo = w

    # per-core per-class node lists
    node_lists = {}  # (core, W) -> ascending node ids
    for c in range(NCORES):
        nlo, nhi = c * chunk, min(N, (c + 1) * chunk)
        d = deg[nlo:nhi]
        lo = 0
        for w in classes:
            sel = np.nonzero((d > lo) & (d <= w))[0] + nlo
            node_lists[(c, w)] = sel
            lo = w

    # identical per-class capacity on every core, in whole 128-node tiles
    caps = {}
    for w in classes:
        cap = max(len(node_lists[(c, w)]) for c in range(NCORES))
        caps[w] = ((cap + P - 1) // P) * P

    n_class_rows = sum(caps.values())
    deg0_max = max(
        ((min(N, (c + 1) * chunk) - c * chunk) - sum(len(node_lists[(c, w)]) for w in classes))
        for c in range(NCORES)
    )
    R = ((n_class_rows + max(deg0_max, 0) + P - 1) // P) * P
    T = R // P

    # perm order per core + global row index of every node
    perm = np.full((NCORES, R), -1, dtype=np.int64)
    row_of = np.full(N, -1, dtype=np.int64)
    base = {}
    b = 0
    for w in classes:
        base[w] = b
        b += caps[w]
    for c in range(NCORES):
        nlo, nhi = c * chunk, min(N, (c + 1) * chunk)
        for w in classes:
            nl = node_lists[(c, w)]
            perm[c, base[w]:base[w] + len(nl)] = nl
            row_of[nl] = c * R + base[w] + np.arange(len(nl))
        deg0 = np.nonzero(deg[nlo:nhi] == 0)[0] + nlo
        perm[c, n_class_rows:n_class_rows + len(deg0)] = deg0
        # deg0 rows need correct x (they may be gathered as src)
        row_of[deg0] = c * R + n_class_rows + np.arange(len(deg0))

    # gather index + bias tables, per class, per core
    idx_tabs = {}   # (c, w) -> [capW, w] int32 rows into global qv table
    bias_tabs = {}
    for c in range(NCORES):
        for w in classes:
            cap = caps[w]
            it = np.zeros((cap, w), dtype=np.int32)
            bt = np.full((cap, w), NEG, dtype=np.float32)
            nl = node_lists[(c, w)]
            for i, g in enumerate(nl):
                d = int(deg[g])
                srcs = src_sorted[starts[g]:starts[g] + d]
                it[i, :d] = row_of[srcs].astype(np.int32)
                bt[i, :d] = 0.0
            idx_tabs[(c, w)] = it
            bias_tabs[(c, w)] = bt

    lay = Layout()
    lay.N, lay.R, lay.T, lay.chunk = N, R, T, chunk
    lay.classes, lay.caps, lay.base = classes, caps, base
    lay.n_class_tiles = n_class_rows // P
    lay.perm, lay.row_of = perm, row_of
    lay.idx_tabs, lay.bias_tabs = idx_tabs, bias_tabs
    return lay


def host_inputs(lay, x, qkv_w, qkv_b, out_w, out_b, out_w_last, out_b_last):
    """Build the per-core in_maps."""
    x = np.asarray(x, dtype=np.float32)
    nclass = out_w_last.shape[1]
    wc = np.zeros((L, D, 3 * D), dtype=np.float32)
    bc = np.zeros((L, P, 3 * D), dtype=np.float32)
    wo = np.zeros((L, D, D), dtype=np.float32)
    bo = np.zeros((L, P, D), dtype=np.float32)
    for l in range(L):
        wq, wk, wv = qkv_w[l, 0], qkv_w[l, 1], qkv_w[l, 2]
        bq, bk, bv = qkv_b[l, 0], qkv_b[l, 1], qkv_b[l, 2]
        wc[l] = np.concatenate([wq, wk * SCALE, wv], axis=1)
        bcl = np.concatenate([bq, bk * SCALE, bv])
        bc[l] = np.tile(bcl[None, :], (P, 1))
        if l < L - 1:
            wo[l] = out_w[l]
            bo[l] = np.tile(out_b[l][None, :], (P, 1))
        else:
            wo[l, :, :nclass] = out_w_last
            bo[l, :, :nclass] = np.tile(out_b_last[None, :], (P, 1))

    in_maps = []
    for c in range(NCORES):
        m = {
            "x0": np.where(
                (lay.perm[c] >= 0)[:, None], x[np.maximum(lay.perm[c], 0)], 0.0
            ).astype(np.float32),
            "wc": wc, "bc": bc, "wo": wo, "bo": bo,
        }
        for w in lay.classes:
            m[f"idx{w}"] = lay.idx_tabs[(c, w)]
            m[f"bias{w}"] = lay.bias_tabs[(c, w)]
        in_maps.append(m)
    return in_maps


def host_output(lay, outs, nclass):
    """Invert the perm: outs is list of [R, nclass] per core."""
    full = np.zeros((lay.N, nclass), dtype=np.float32)
    for c in range(NCORES):
        real = lay.perm[c] >= 0
        full[lay.perm[c][real]] = outs[c][real]
    return full


# ----------------------------------------------------------------------------
# Device program
# ----------------------------------------------------------------------------

def build_nc(lay, nclass):
    R, T = lay.R, lay.T
    nc = bacc.Bacc(trn_type="TRN2", num_devices=NCORES)

    x0 = nc.dram_tensor("x0", [R, D], FP, kind="ExternalInput")
    wc = nc.dram_tensor("wc", [L, D, 3 * D], FP, kind="ExternalInput")
    bc = nc.dram_tensor("bc", [L, P, 3 * D], FP, kind="ExternalInput")
    wo = nc.dram_tensor("wo", [L, D, D], FP, kind="ExternalInput")
    bo = nc.dram_tensor("bo", [L, P, D], FP, kind="ExternalInput")
    idx_d, bias_d = {}, {}
    for w in lay.classes:
        cap = lay.caps[w]
        idx_d[w] = nc.dram_tensor(f"idx{w}", [cap, w], I32, kind="ExternalInput")
        bias_d[w] = nc.dram_tensor(f"bias{w}", [cap, w], FP, kind="ExternalInput")
    out_ext = nc.dram_tensor("out", [R, nclass], FP, kind="ExternalOutput")

    qv_slice = nc.dram_tensor("qv_slice", [R, 2 * D], FP, kind="Internal")
    qv_full = nc.dram_tensor(
        "qv_full", [NCORES * R, 2 * D], FP, kind="Internal", addr_space="Shared"
    )
    rg = [list(range(NCORES))]

    with tile.TileContext(nc) as tc:
        with (
            tc.tile_pool(name="const", bufs=1) as cpool,
            tc.tile_pool(name="persist", bufs=1) as ppool,
            tc.tile_pool(name="proj", bufs=3) as projpool,
            tc.tile_pool(name="work", bufs=2) as wpool,
            tc.tile_pool(name="small", bufs=3) as spool,
            tc.tile_pool(name="psum", bufs=2, space="PSUM") as pspool,
            tc.tile_pool(name="psum_o", bufs=2, space="PSUM") as pspool_o,
        ):
            ident = cpool.tile([P, P], FP, tag="ident", name="ident")
            make_identity(nc, ident[:])
            wc_sb = cpool.tile([P, L * 3 * D], FP, tag="wc", name="wc")
            nc.sync.dma_start(wc_sb[:].rearrange("k (l n) -> k l n", l=L), wc[:].rearrange("l k n -> k l n"))
            bc_sb = cpool.tile([P, L * 3 * D], FP, tag="bc", name="bc")
            nc.sync.dma_start(bc_sb[:].rearrange("p (l n) -> p l n", l=L), bc[:].rearrange("l p n -> p l n"))
            wo_sb = cpool.tile([P, L * D], FP, tag="wo", name="wo")
            nc.sync.dma_start(wo_sb[:].rearrange("k (l n) -> k l n", l=L), wo[:].rearrange("l k n -> k l n"))
            bo_sb = cpool.tile([P, L * D], FP, tag="bo", name="bo")
            nc.sync.dma_start(bo_sb[:].rearrange("p (l n) -> p l n", l=L), bo[:].rearrange("l p n -> p l n"))

            idx_sb, bias_sb = {}, {}
            for w in lay.classes:
                tw = lay.caps[w] // P
                idx_sb[w] = cpool.tile([P, tw * w], I32, tag=f"idx{w}", name=f"idx{w}")
                nc.sync.dma_start(
                    idx_sb[w][:].rearrange("p (t w) -> p t w", w=w),
                    idx_d[w][:].rearrange("(t p) w -> p t w", p=P),
                )
                bias_sb[w] = cpool.tile([P, tw * w], FP, tag=f"bias{w}", name=f"bias{w}")
                nc.sync.dma_start(
                    bias_sb[w][:].rearrange("p (t w) -> p t w", w=w),
                    bias_d[w][:].rearrange("(t p) w -> p t w", p=P),
                )

            x_sb = ppool.tile([P, T * D], FP, tag="x", name="x")
            nc.sync.dma_start(x_sb[:].rearrange("p (t f) -> p t f", f=D), x0[:].rearrange("(t p) f -> p t f", p=P))
            k_sb = ppool.tile([P, T * D], FP, tag="k", name="k")
            tc.strict_bb_all_engine_barrier()

            for l in range(L):
                # ---- QKV projection for own rows ----
                for t in range(T):
                    xT_ps = pspool.tile([P, P], FP, tag="xT", name="xT")
                    nc.tensor.transpose(
                        xT_ps[:], x_sb[:, t * D:(t + 1) * D], ident[:]
                    )
                    xT = projpool.tile([P, P], FP, tag="xT_sb", name="xT_sb")
                    nc.vector.tensor_copy(xT[:], xT_ps[:])
                    qkv_ps = pspool.tile([P, 3 * D], FP, tag="qkv", name="qkv")
                    nc.tensor.matmul(
                        qkv_ps[:], lhsT=xT[:],
                        rhs=wc_sb[:, l * 3 * D:(l + 1) * 3 * D],
                        start=True, stop=True,
                    )
                    qv_st = projpool.tile([P, 2 * D], FP, tag="qv_st", name="qv_st")
                    bofs = l * 3 * D
                    nc.vector.tensor_tensor(
                        out=qv_st[:, 0:D], in0=qkv_ps[:, 0:D],
                        in1=bc_sb[:, bofs:bofs + D], op=OP.add,
                    )
                    nc.vector.tensor_tensor(
                        out=qv_st[:, D:2 * D], in0=qkv_ps[:, 2 * D:3 * D],
                        in1=bc_sb[:, bofs + 2 * D:bofs + 3 * D], op=OP.add,
                    )
                    nc.vector.tensor_tensor(
                        out=k_sb[:, t * D:(t + 1) * D], in0=qkv_ps[:, D:2 * D],
                        in1=bc_sb[:, bofs + D:bofs + 2 * D], op=OP.add,
                    )
                    nc.sync.dma_start(qv_slice[t * P:(t + 1) * P, :], qv_st[:])

                # ---- exchange qv ----
                nc.gpsimd.collective_compute(
                    "AllGather", OP.bypass, replica_groups=rg,
                    ins=[qv_slice[:, :]], outs=[qv_full[:, :]],
                )
                tc.strict_bb_all_engine_barrier()

                # ---- per-class edge softmax + aggregation + out projection ----
                nt = 0
                for w in lay.classes:
                    tw = lay.caps[w] // P
                    for t in range(tw):
                        qv_g = wpool.tile([P, w * 2 * D], FP, tag="qvg", name=f"qvg{w}")
                        # HW indirect DMA consumes ONE offset per partition and
                        # reads the dest's free size contiguously from that row
                        # (interp's per-chunk-offset semantics do not hold), so
                        # issue one gather per edge slot.
                        for ws in range(w):
                            nc.gpsimd.indirect_dma_start(
                                out=qv_g[:, ws * 2 * D:(ws + 1) * 2 * D],
                                out_offset=None,
                                in_=qv_full[:, :],
                                in_offset=IndirectOffsetOnAxis(
                                    ap=idx_sb[w][:, t * w + ws:t * w + ws + 1], axis=0
                                ),
                            )
                        q_ap = qv_g[:].rearrange("p (w c) -> p w c", c=2 * D)[:, :, 0:D]
                        v_ap = qv_g[:].rearrange("p (w c) -> p w c", c=2 * D)[:, :, D:2 * D]
                        k_ap = (
                            k_sb[:, nt * D:(nt + 1) * D]
                            .unsqueeze(1).to_broadcast([P, w, D])
                        )
                        tmp = wpool.tile([P, w * D], FP, tag="tmp", name=f"tmp{w}")
                        nc.vector.tensor_tensor(
                            out=tmp[:].rearrange("p (w f) -> p w f", f=D),
                            in0=q_ap, in1=k_ap, op=OP.mult,
                        )
                        s = spool.tile([P, w * H], FP, tag="s", name=f"s{w}")
                        nc.vector.reduce_sum(
                            s[:].rearrange("p (w h) -> p w h", h=H),
                            tmp[:].rearrange("p (w h d) -> p w h d", h=H, d=HD),
                            axis=AX.X,
                        )
                        bia = (
                            bias_sb[w][:, t * w:(t + 1) * w]
                            .unsqueeze(2).to_broadcast([P, w, H])
                        )
                        nc.vector.tensor_tensor(
                            out=s[:].rearrange("p (w h) -> p w h", h=H),
                            in0=s[:].rearrange("p (w h) -> p w h", h=H),
                            in1=bia, op=OP.add,
                        )
                        smax = spool.tile([P, H], FP, tag="smax", name="smax")
                        nc.vector.reduce_max(
                            smax[:],
                            s[:].rearrange("p (w h) -> p h w", h=H),
                            axis=AX.X,
                        )
                        ex = spool.tile([P, w * H], FP, tag="ex", name=f"ex{w}")
                        nc.vector.tensor_tensor(
                            out=ex[:].rearrange("p (w h) -> p w h", h=H),
                            in0=s[:].rearrange("p (w h) -> p w h", h=H),
                            in1=smax[:].unsqueeze(1).to_broadcast([P, w, H]),
                            op=OP.subtract,
                        )
                        nc.scalar.activation(
                            out=ex[:], in_=ex[:],
                            func=mybir.ActivationFunctionType.Exp,
                        )
                        denom = spool.tile([P, H], FP, tag="denom", name="denom")
                        nc.vector.reduce_sum(
                            denom[:],
                            ex[:].rearrange("p (w h) -> p h w", h=H),
                            axis=AX.X,
                        )
                        rec = spool.tile([P, H], FP, tag="rec", name="rec")
                        nc.vector.reciprocal(rec[:], denom[:])
                        probs = spool.tile([P, w * H], FP, tag="probs", name=f"probs{w}")
                        nc.vector.tensor_tensor(
                            out=probs[:].rearrange("p (w h) -> p w h", h=H),
                            in0=ex[:].rearrange("p (w h) -> p w h", h=H),
                            in1=rec[:].unsqueeze(1).to_broadcast([P, w, H]),
                            op=OP.mult,
                        )
                        nc.vector.tensor_tensor(
                            out=tmp[:].rearrange("p (w h d) -> p w h d", h=H, d=HD),
                            in0=v_ap.rearrange("p w (h d) -> p w h d", h=H),
                            in1=probs[:].rearrange("p (w h) -> p w h", h=H).unsqueeze(3)
                                .to_broadcast([P, w, H, HD]),
                            op=OP.mult,
                        )
                        agg = projpool.tile([P, D], FP, tag="agg", name="agg")
                        nc.vector.reduce_sum(
                            agg[:],
                            tmp[:].rearrange("p (w f) -> p f w", f=D),
                            axis=AX.X,
                        )
                        # fused output projection for this node tile
                        aT_ps = pspool.tile([P, P], FP, tag="aT", name="aT")
                        nc.tensor.transpose(aT_ps[:], agg[:], ident[:])
                        aT = projpool.tile([P, P], FP, tag="aT_sb", name="aT_sb")
                        nc.vector.tensor_copy(aT[:], aT_ps[:])
                        o_ps = pspool_o.tile([P, D], FP, tag="o", name="o")
                        nc.tensor.matmul(
                            o_ps[:], lhsT=aT[:], rhs=wo_sb[:, l * D:(l + 1) * D],
                            start=True, stop=True,
                        )
                        if l < L - 1:
                            nc.vector.tensor_tensor(
                                out=x_sb[:, nt * D:(nt + 1) * D], in0=o_ps[:],
                                in1=bo_sb[:, l * D:(l + 1) * D], op=OP.add,
                            )
                        else:
                            o_sb = projpool.tile([P, nclass], FP, tag="o_sb", name="o_sb")
                            nc.vector.tensor_tensor(
                                out=o_sb[:], in0=o_ps[:, 0:nclass],
                                in1=bo_sb[:, l * D:l * D + nclass], op=OP.add,
                            )
                            nc.sync.dma_start(
                                out_ext[nt * P:(nt + 1) * P, :], o_sb[:]
                            )
                        nt += 1
                # tail tiles (deg-0 + padding rows): out = bias only
                for t in range(lay.n_class_tiles, T):
                    if l < L - 1:
                        nc.vector.tensor_copy(
                            x_sb[:, t * D:(t + 1) * D], bo_sb[:, l * D:(l + 1) * D]
                        )
                    else:
                        o_sb = projpool.tile([P, nclass], FP, tag="o_sb", name="o_sb")
                        nc.vector.tensor_copy(
                            o_sb[:], bo_sb[:, l * D:l * D + nclass]
                        )
                        nc.sync.dma_start(out_ext[t * P:(t + 1) * P, :], o_sb[:])
    nc.compile()
    return nc


# ----------------------------------------------------------------------------
# Entry point
# ----------------------------------------------------------------------------

_trace = [False]  # test.py can flip this to profile


def kernel(x, src, dst, qkv_w, qkv_b, out_w, out_b, out_w_last, out_b_last):
    x = np.asarray(x, dtype=np.float32)
    lay = build_layout(np.asarray(src), np.asarray(dst), x.shape[0])
    nclass = np.asarray(out_w_last).shape[1]
    in_maps = host_inputs(
        lay, x, np.asarray(qkv_w, dtype=np.float32),
        np.asarray(qkv_b, dtype=np.float32), np.asarray(out_w, dtype=np.float32),
        np.asarray(out_b, dtype=np.float32), np.asarray(out_w_last, dtype=np.float32),
        np.asarray(out_b_last, dtype=np.float32),
    )
    nc = build_nc(lay, nclass)
    res = run_bass_kernel_spmd(
        nc, in_maps, core_ids=list(range(NCORES)), trace=_trace[0]
    )
    kernel.last_results = res
    outs = [res.results[c]["out"] for c in range(NCORES)]
    return host_output(lay, outs, nclass)

